# revision 1
# baseline (speedup 1.0000x reference)
"""TransformerConv graph attention (IntraGraphAttention) on 8 Trainium2 cores.

Sharding: dst-node partition across 8 cores (degree-sorted, block-cyclic for
load balance); projection weights replicated; each core computes the full k/v
table on-device (replicated GEMMs, no collectives) and gathers k/v rows for
its own edges via indirect DMA.
"""

import sys

sys.path.insert(0, "/opt/trn_rl_repo")

import numpy as np

import concourse.bass as bass
import concourse.mybir as mybir
import concourse.tile as tile
from concourse.bass import AP, IndirectOffsetOnAxis
from concourse.bass_utils import run_bass_kernel_spmd

N = 50000
E = 1_600_000
D = 128
H = 2
C = 32
HC = H * C  # 64
NCORES = 8
P = 128
NPAD = 50176          # 392 tiles of 128; per-core 49 tiles
TILES_PER_CORE = NPAD // P // NCORES  # 49
NODES_PER_CORE = TILES_PER_CORE * P   # 6272
KV_ROWS = 50048       # 391 x-tiles written by phase A
DUMMY = 50000         # kv row used for padded slots (zeroed on device)
NEG = -1.0e30


# ---------------------------------------------------------------------------
# walrus in this container rejects instructions carrying >1 sync-wait; split
# extras onto same-engine NOPs (and the tail drain into single-wait drains).
def _patch_tile(tile_mod):
    from concourse.vector_clock import ScopedClock

    def _drain_and_barrier(self, tick_clock, wait_clock):
        nc = self.nc
        drain_inst = nc.sync.drain()
        wait_clock.add_sem_waits(
            drain_inst.ins, ScopedClock({None: tick_clock.global_clock})
        )
        si = drain_inst.ins.sync_info
        if si is not None and si.on_wait and len(si.on_wait) > 1:
            waits = list(si.on_wait)
            si.on_wait = waits[:1]
            for w in waits[1:]:
                extra = nc.sync.drain()
                esi = extra.ins.sync_info
                if esi is None:
                    extra.ins.sync_info = mybir.SyncInfo(on_wait=[w], on_update=[])
                else:
                    esi.on_wait = [w]
        nc.all_engine_barrier()
        assert self.sems is not None
        popped = nc._tile_sem_poison_stack.pop()
        assert popped is self._sem_poison
        nc.clear_and_free_semaphores(list(self.sems.allocated().values()))
        nc.all_engine_barrier()

    tile_mod.TileContext._drain_and_barrier = _drain_and_barrier


def _split_multi_waits(nc):
    f = nc.m.functions[0]
    for bb in f.blocks:
        out = []
        for inst in bb.instructions:
            si = inst.sync_info
            waits = list(si.on_wait) if (si is not None and si.on_wait) else []
            if len(waits) > 1:
                eng = inst.engine
                for w in waits[:-1]:
                    bi = nc.engines[eng].nop(nofuse=True)
                    mi = bi.ins
                    for b2 in f.blocks:
                        if mi in b2.instructions:
                            b2.instructions.remove(mi)
                            break
                    esi = mi.sync_info
                    if esi is None:
                        mi.sync_info = mybir.SyncInfo(on_wait=[w], on_update=[])
                    else:
                        esi.on_wait = [w]
                    out.append(mi)
                si.on_wait = waits[-1:]
            out.append(inst)
        bb.instructions[:] = out


_patch_tile(tile)


# ---------------------------------------------------------------------------
def _build_program(k_per_tile):
    """One SPMD program; per-core data differs but shapes are identical."""
    f32 = mybir.dt.float32
    SK = int(sum(k_per_tile))
    nc = bass.Bass("TRN2")
    xT = nc.dram_tensor("xT", [P, KV_ROWS], f32, kind="ExternalInput")
    xpT = nc.dram_tensor("xpT", [P, NODES_PER_CORE], f32, kind="ExternalInput")
    wkv = nc.dram_tensor("wkv", [D, 2 * HC], f32, kind="ExternalInput")
    wqs = nc.dram_tensor("wqs", [D, 2 * HC], f32, kind="ExternalInput")
    bkv = nc.dram_tensor("bkv", [P, 2 * HC], f32, kind="ExternalInput")
    bqs = nc.dram_tensor("bqs", [P, 2 * HC], f32, kind="ExternalInput")
    idxb = nc.dram_tensor("idxb", [P, SK], mybir.dt.int32, kind="ExternalInput")
    mskb = nc.dram_tensor("mskb", [P, SK], f32, kind="ExternalInput")
    outd = nc.dram_tensor("out", [NODES_PER_CORE, HC], f32, kind="ExternalOutput")
    kvt = nc.dram_tensor("kvt", [KV_ROWS, D], f32, kind="Internal")

    EXP = mybir.ActivationFunctionType.Exp
    RELU = mybir.ActivationFunctionType.Relu
    MIN = mybir.AluOpType.min
    MULT = mybir.AluOpType.mult
    ADD = mybir.AluOpType.add
    MAX = mybir.AluOpType.max
    AXX = mybir.AxisListType.X

    with tile.TileContext(nc) as tc:
        with (
            tc.tile_pool(name="const", bufs=1) as cpool,
            tc.tile_pool(name="qs", bufs=1) as qpool,
            tc.tile_pool(name="pa", bufs=3) as pa,
            tc.tile_pool(name="psA", bufs=4, space="PSUM") as psA,
            tc.tile_pool(name="pc", bufs=2) as pc,
            tc.tile_pool(name="pcs", bufs=2) as pcs,
        ):
            wkv_sb = cpool.tile([D, 2 * HC], f32)
            wqs_sb = cpool.tile([D, 2 * HC], f32)
            bkv_sb = cpool.tile([P, 2 * HC], f32)
            bqs_sb = cpool.tile([P, 2 * HC], f32)
            zero_sb = cpool.tile([P, D], f32)
            nc.sync.dma_start(out=wkv_sb[:], in_=wkv[:, :])
            nc.sync.dma_start(out=wqs_sb[:], in_=wqs[:, :])
            nc.sync.dma_start(out=bkv_sb[:], in_=bkv[:, :])
            nc.sync.dma_start(out=bqs_sb[:], in_=bqs[:, :])
            nc.vector.memset(zero_sb[:], 0.0)
            qs_sb = qpool.tile([P, NODES_PER_CORE], f32)

            # ---- phases A (kv table) and B (q|skip, kept in SBUF) ----------
            def proj_slab(src, c0, w, wt, bt, sink):
                xt = pa.tile([P, 512], f32, tag="xt")
                mn = pa.tile([P, 512], f32, tag="mn")
                yy = pa.tile([P, 512], f32, tag="yy")
                nc.sync.dma_start(out=xt[:, :w], in_=src[:, c0 : c0 + w])
                # y = elu(x)+1 = relu(x) + exp(min(x,0)); the +1 is folded
                # into the bias tiles host-side (b - colsum(W)).
                nc.vector.tensor_scalar(
                    out=mn[:, :w], in0=xt[:, :w], scalar1=0.0, scalar2=None, op0=MIN
                )
                nc.scalar.activation(out=yy[:, :w], in_=xt[:, :w], func=RELU)
                nc.scalar.activation(out=mn[:, :w], in_=mn[:, :w], func=EXP)
                nc.vector.tensor_add(out=yy[:, :w], in0=yy[:, :w], in1=mn[:, :w])
                for j in range(w // P):
                    ps = psA.tile([P, 2 * HC], f32, tag="ps")
                    nc.tensor.matmul(
                        out=ps[:],
                        lhsT=yy[:, j * P : (j + 1) * P],
                        rhs=wt[:],
                        start=True,
                        stop=True,
                    )
                    sink(c0 + j * P, ps, bt)

            def kv_sink(r0, ps, bt):
                kv_sb = pa.tile([P, 2 * HC], f32, tag="kvsb")
                nc.vector.tensor_add(out=kv_sb[:], in0=ps[:], in1=bt[:])
                nc.sync.dma_start(out=kvt[r0 : r0 + P, :], in_=kv_sb[:])

            def qs_sink(r0, ps, bt):
                nc.vector.tensor_add(
                    out=qs_sb[:, r0 : r0 + 2 * HC], in0=ps[:], in1=bt[:]
                )

            c0 = 0
            while c0 < KV_ROWS:
                w = min(512, KV_ROWS - c0)
                proj_slab(xT, c0, w, wkv_sb, bkv_sb, kv_sink)
                c0 += w
            c0 = 0
            while c0 < NODES_PER_CORE:
                w = min(512, NODES_PER_CORE - c0)
                proj_slab(xpT, c0, w, wqs_sb, bqs_sb, qs_sink)
                c0 += w
            # zero the dummy row so padded slots contribute v = 0
            nc.sync.dma_start(out=kvt[DUMMY : DUMMY + 1, :], in_=zero_sb[:1, :])

            # ---- phase C: gather + segment softmax + weighted sum ----------
            ot = 0
            for t in range(TILES_PER_CORE):
                K = int(k_per_tile[t])
                idx_sb = pc.tile([P, K], mybir.dt.int32, tag="idx")
                msk_sb = pc.tile([P, K], f32, tag="msk")
                g_sb = pc.tile([P, K * D], f32, tag="g")
                nc.sync.dma_start(out=idx_sb[:], in_=idxb[:, ot : ot + K])
                nc.sync.dma_start(out=msk_sb[:], in_=mskb[:, ot : ot + K])
                for k in range(K):
                    nc.gpsimd.indirect_dma_start(
                        out=g_sb[:, k * D : (k + 1) * D],
                        out_offset=None,
                        in_=kvt[:, :],
                        in_offset=IndirectOffsetOnAxis(
                            ap=idx_sb[:, k : k + 1], axis=0
                        ),
                    )
                ga = g_sb[:]
                pstr = ga.ap[0]
                goff = ga.offset
                # prod[p, h, e, c] = k_g[p, e, h, c] * q[p, h, c]
                prod = pcs.tile([P, 2 * K * C], f32, tag="prod")
                qsl = qs_sb[:, t * P : t * P + HC]
                nc.vector.tensor_tensor(
                    out=AP(
                        prod[:].tensor,
                        prod[:].offset,
                        [prod[:].ap[0], [C, K], [K * C, H], [1, C]],
                    ),
                    in0=AP(ga.tensor, goff, [pstr, [D, K], [C, H], [1, C]]),
                    in1=AP(qsl.tensor, qsl.offset, [qsl.ap[0], [0, K], [C, H], [1, C]]),
                    op=MULT,
                )
                alpha = pcs.tile([P, 2 * K], f32, tag="alpha")
                pv = prod[:]
                nc.vector.tensor_reduce(
                    out=alpha[:],
                    in_=AP(pv.tensor, pv.offset, [pv.ap[0], [K * C, H], [C, K], [1, C]]),
                    axis=AXX,
                    op=ADD,
                )
                mv = msk_sb[:]
                nc.vector.tensor_tensor(
                    out=alpha[:],
                    in0=alpha[:],
                    in1=AP(mv.tensor, mv.offset, [mv.ap[0], [0, H], [1, K]]),
                    op=ADD,
                )
                m_sb = pcs.tile([P, H], f32, tag="m")
                av = alpha[:]
                nc.vector.tensor_reduce(
                    out=m_sb[:],
                    in_=AP(av.tensor, av.offset, [av.ap[0], [K, H], [1, K]]),
                    axis=AXX,
                    op=MAX,
                )
                negm = pcs.tile([P, H], f32, tag="negm")
                nc.vector.tensor_scalar_mul(negm[:], m_sb[:], -1.0)
                ex = pcs.tile([P, 2 * K], f32, tag="ex")
                for h in range(H):
                    nc.scalar.activation(
                        out=ex[:, h * K : (h + 1) * K],
                        in_=alpha[:, h * K : (h + 1) * K],
                        func=EXP,
                        bias=negm[:, h : h + 1],
                        scale=1.0,
                    )
                den = pcs.tile([P, H], f32, tag="den")
                ev = ex[:]
                nc.vector.tensor_reduce(
                    out=den[:],
                    in_=AP(ev.tensor, ev.offset, [ev.ap[0], [K, H], [1, K]]),
                    axis=AXX,
                    op=ADD,
                )
                rden = pcs.tile([P, H], f32, tag="rden")
                nc.vector.reciprocal(rden[:], den[:])
                for h in range(H):
                    nc.vector.tensor_scalar(
                        out=ex[:, h * K : (h + 1) * K],
                        in0=ex[:, h * K : (h + 1) * K],
                        scalar1=rden[:, h : h + 1],
                        scalar2=None,
                        op0=MULT,
                    )
                # prod2[p, h, c, e] = v_g[p, e, h, c] * w[p, h, e]
                prod2 = pcs.tile([P, 2 * K * C], f32, tag="prod2")
                p2 = prod2[:]
                nc.vector.tensor_tensor(
                    out=AP(
                        p2.tensor, p2.offset, [p2.ap[0], [1, K], [K * C, H], [K, C]]
                    ),
                    in0=AP(ga.tensor, goff + HC, [pstr, [D, K], [C, H], [1, C]]),
                    in1=AP(ev.tensor, ev.offset, [ev.ap[0], [1, K], [K, H], [0, C]]),
                    op=MULT,
                )
                att = pcs.tile([P, HC], f32, tag="att")
                nc.vector.tensor_reduce(
                    out=att[:],
                    in_=AP(
                        p2.tensor, p2.offset, [p2.ap[0], [K * C, H], [K, C], [1, K]]
                    ),
                    axis=AXX,
                    op=ADD,
                )
                outt = pcs.tile([P, HC], f32, tag="outt")
                nc.vector.tensor_add(
                    out=outt[:], in0=att[:], in1=qs_sb[:, t * P + HC : (t + 1) * P]
                )
                nc.sync.dma_start(out=outd[t * P : (t + 1) * P, :], in_=outt[:])
                ot += K
    _split_multi_waits(nc)
    return nc


# ---------------------------------------------------------------------------
def kernel(x, edge_index, Wq, bq, Wk, bk, Wv, bv, Wskip, bskip, _trace=False):
    x = np.asarray(x, np.float32)
    src = np.asarray(edge_index[0], np.int64)
    dst = np.asarray(edge_index[1], np.int64)
    Wq, bq, Wk, bk = map(np.asarray, (Wq, bq, Wk, bk))
    Wv, bv, Wskip, bskip = map(np.asarray, (Wv, bv, Wskip, bskip))

    s = 1.0 / np.sqrt(np.float32(C))
    wkv = np.concatenate([Wk, Wv], 1).astype(np.float32)            # [128,128]
    wqs = np.concatenate([Wq * s, Wskip], 1).astype(np.float32)
    bkv = np.concatenate([bk - Wk.sum(0), bv - Wv.sum(0)]).astype(np.float32)
    bqs = np.concatenate(
        [(bq - Wq.sum(0)) * s, bskip - Wskip.sum(0)]
    ).astype(np.float32)
    bkv_t = np.tile(bkv[None, :], (P, 1))
    bqs_t = np.tile(bqs[None, :], (P, 1))

    # CSR over dst
    deg = np.bincount(dst, minlength=N)
    order = np.argsort(dst, kind="stable")
    src_sorted = src[order]
    rowptr = np.zeros(N + 1, np.int64)
    np.cumsum(deg, out=rowptr[1:])

    # degree-sorted nodes, block-cyclic deal of 128-blocks to cores
    nodes_sorted = np.argsort(-deg, kind="stable")
    nodes_pad = np.concatenate([nodes_sorted, np.full(NPAD - N, -1, np.int64)])
    blocks = nodes_pad.reshape(-1, P)                       # [392, 128]
    core_nodes = [
        blocks[c::NCORES].reshape(-1) for c in range(NCORES)
    ]  # each [6272]

    # per-tile K unified across cores
    k_per_tile = np.zeros(TILES_PER_CORE, np.int64)
    for c in range(NCORES):
        d = np.where(core_nodes[c] >= 0, deg[np.maximum(core_nodes[c], 0)], 0)
        k_per_tile = np.maximum(k_per_tile, d.reshape(TILES_PER_CORE, P).max(1))
    k_per_tile = np.maximum(k_per_tile, 2)
    SK = int(k_per_tile.sum())

    xT = np.zeros((P, KV_ROWS), np.float32)
    xT[:, :N] = x.T
    in_maps = []
    for c in range(NCORES):
        nodes = core_nodes[c]
        xpT = np.zeros((P, NODES_PER_CORE), np.float32)
        real = nodes >= 0
        xpT[:, real] = x[nodes[real]].T
        idxb = np.full((P, SK), DUMMY, np.int32)
        mskb = np.full((P, SK), NEG, np.float32)
        ot = 0
        for t in range(TILES_PER_CORE):
            K = int(k_per_tile[t])
            tn = nodes[t * P : (t + 1) * P]
            for p in range(P):
                n = tn[p]
                if n < 0:
                    continue
                dg = int(deg[n])
                if dg:
                    idxb[p, ot : ot + dg] = src_sorted[rowptr[n] : rowptr[n] + dg]
                    mskb[p, ot : ot + dg] = 0.0
            ot += K
        in_maps.append(
            {
                "xT": xT,
                "xpT": xpT,
                "wkv": wkv,
                "wqs": wqs,
                "bkv": bkv_t,
                "bqs": bqs_t,
                "idxb": idxb,
                "mskb": mskb,
            }
        )

    nc = _build_program(k_per_tile)
    res = run_bass_kernel_spmd(nc, in_maps, core_ids=list(range(NCORES)))
    if _trace:
        # no NTFF hook in this container: report warm-NEFF wall time of a
        # second dispatch (upper bound: includes axon transfer + dispatch)
        import time as _time

        t0 = _time.time()
        res = run_bass_kernel_spmd(nc, in_maps, core_ids=list(range(NCORES)))
        kernel.last_wall_ns = int((_time.time() - t0) * 1e9)
    out_full = np.zeros((N, HC), np.float32)
    for c in range(NCORES):
        nodes = core_nodes[c]
        o = res.results[c]["out"]
        real = nodes >= 0
        out_full[nodes[real]] = o[real]
    kernel.last_exec_time_ns = res.exec_time_ns
    return out_full



# revision 2
# speedup vs baseline: 5.9494x; 5.9494x over previous
"""TransformerConv graph attention (IntraGraphAttention) on 8 Trainium2 cores.

Sharding: nodes are degree-sorted and dealt block-cyclically (128-blocks) to 8
cores; each core receives ONLY its own x shard (bf16, transposed), computes
k|v projections for its shard, and the full k/v table is assembled on-device
with an AllGather over NeuronLink. Edges are sharded by dst node; per-edge
source positions ship as uint16, padding masks are built on-device from
per-node degrees. Output returns bf16. This keeps host<->device traffic at
~24MB total (vs ~260MB for a replicated-x design), which dominates wall time
through the PJRT transport.
"""

import sys

sys.path.insert(0, "/opt/trn_rl_repo")

import numpy as np
import ml_dtypes

import concourse.bass as bass
import concourse.mybir as mybir
import concourse.tile as tile
from concourse.bass import AP, IndirectOffsetOnAxis
from concourse.bass_utils import run_bass_kernel_spmd

N = 50000
E = 1_600_000
D = 128
H = 2
C = 32
HC = H * C  # 64
NCORES = 8
P = 128
NPAD = 50176          # 392 tiles of 128; per-core 49 tiles
TILES_PER_CORE = NPAD // P // NCORES  # 49
NODES_PER_CORE = TILES_PER_CORE * P   # 6272
NEG = -1.0e30

f32 = mybir.dt.float32
bf16 = mybir.dt.bfloat16
i32 = mybir.dt.int32
u16 = mybir.dt.uint16


# ---------------------------------------------------------------------------
# walrus in this container rejects instructions carrying >1 sync-wait; split
# extras onto same-engine NOPs (and the tail drain into single-wait drains).
def _patch_tile(tile_mod):
    from concourse.vector_clock import ScopedClock

    def _drain_and_barrier(self, tick_clock, wait_clock):
        nc = self.nc
        drain_inst = nc.sync.drain()
        wait_clock.add_sem_waits(
            drain_inst.ins, ScopedClock({None: tick_clock.global_clock})
        )
        si = drain_inst.ins.sync_info
        if si is not None and si.on_wait and len(si.on_wait) > 1:
            waits = list(si.on_wait)
            si.on_wait = waits[:1]
            for w in waits[1:]:
                extra = nc.sync.drain()
                esi = extra.ins.sync_info
                if esi is None:
                    extra.ins.sync_info = mybir.SyncInfo(on_wait=[w], on_update=[])
                else:
                    esi.on_wait = [w]
        nc.all_engine_barrier()
        assert self.sems is not None
        popped = nc._tile_sem_poison_stack.pop()
        assert popped is self._sem_poison
        nc.clear_and_free_semaphores(list(self.sems.allocated().values()))
        nc.all_engine_barrier()

    tile_mod.TileContext._drain_and_barrier = _drain_and_barrier


def _split_multi_waits(nc):
    f = nc.m.functions[0]
    for bb in f.blocks:
        out = []
        for inst in bb.instructions:
            si = inst.sync_info
            waits = list(si.on_wait) if (si is not None and si.on_wait) else []
            if len(waits) > 1:
                eng = inst.engine
                for w in waits[:-1]:
                    bi = nc.engines[eng].nop(nofuse=True)
                    mi = bi.ins
                    for b2 in f.blocks:
                        if mi in b2.instructions:
                            b2.instructions.remove(mi)
                            break
                    esi = mi.sync_info
                    if esi is None:
                        mi.sync_info = mybir.SyncInfo(on_wait=[w], on_update=[])
                    else:
                        esi.on_wait = [w]
                    out.append(mi)
                si.on_wait = waits[-1:]
            out.append(inst)
        bb.instructions[:] = out


_patch_tile(tile)


# ---------------------------------------------------------------------------
def _build_program(k_per_tile):
    """One SPMD program; per-core data differs but shapes are identical."""
    SK = int(sum(k_per_tile))
    KMAX = int(max(k_per_tile))
    nc = bass.Bass("TRN2", num_devices=NCORES)
    xcT = nc.dram_tensor("xcT", [P, NODES_PER_CORE], bf16, kind="ExternalInput")
    wkv = nc.dram_tensor("wkv", [D, 2 * HC], bf16, kind="ExternalInput")
    wqs = nc.dram_tensor("wqs", [D, 2 * HC], bf16, kind="ExternalInput")
    bkv = nc.dram_tensor("bkv", [P, 2 * HC], f32, kind="ExternalInput")
    bqs = nc.dram_tensor("bqs", [P, 2 * HC], f32, kind="ExternalInput")
    idxb = nc.dram_tensor("idxb", [P, SK], u16, kind="ExternalInput")
    degt = nc.dram_tensor("degt", [P, TILES_PER_CORE], f32, kind="ExternalInput")
    outd = nc.dram_tensor("out", [NODES_PER_CORE, HC], bf16, kind="ExternalOutput")
    kvsh = nc.dram_tensor("kvsh", [NODES_PER_CORE, D], bf16, kind="Internal")
    kvt = nc.dram_tensor(
        "kvt", [NPAD, D], bf16, kind="Internal", addr_space="Shared"
    )

    EXP = mybir.ActivationFunctionType.Exp
    RELU = mybir.ActivationFunctionType.Relu
    MIN = mybir.AluOpType.min
    MULT = mybir.AluOpType.mult
    ADD = mybir.AluOpType.add
    MAX = mybir.AluOpType.max
    ISGE = mybir.AluOpType.is_ge
    AXX = mybir.AxisListType.X

    with tile.TileContext(nc) as tc:
        with (
            tc.tile_pool(name="const", bufs=1) as cpool,
            tc.tile_pool(name="qs", bufs=1) as qpool,
            tc.tile_pool(name="pa", bufs=3) as pa,
            tc.tile_pool(name="psA", bufs=4, space="PSUM") as psA,
            tc.tile_pool(name="pc", bufs=2) as pc,
            tc.tile_pool(name="pcs", bufs=2) as pcs,
        ):
            wkv_bf = cpool.tile([D, 2 * HC], bf16)
            wqs_bf = cpool.tile([D, 2 * HC], bf16)
            wkv_sb = cpool.tile([D, 2 * HC], f32)
            wqs_sb = cpool.tile([D, 2 * HC], f32)
            bkv_sb = cpool.tile([P, 2 * HC], f32)
            bqs_sb = cpool.tile([P, 2 * HC], f32)
            iota_sb = cpool.tile([P, KMAX], f32)
            nc.sync.dma_start(out=wkv_bf[:], in_=wkv[:, :])
            nc.sync.dma_start(out=wqs_bf[:], in_=wqs[:, :])
            nc.sync.dma_start(out=bkv_sb[:], in_=bkv[:, :])
            nc.sync.dma_start(out=bqs_sb[:], in_=bqs[:, :])
            nc.scalar.copy(out=wkv_sb[:], in_=wkv_bf[:])
            nc.scalar.copy(out=wqs_sb[:], in_=wqs_bf[:])
            nc.gpsimd.iota(
                iota_sb[:], pattern=[[1, KMAX]], base=0, channel_multiplier=0,
                allow_small_or_imprecise_dtypes=True,
            )
            qs_sb = qpool.tile([P, NODES_PER_CORE], f32)
            # full-size edge index (uint16 shipped, int32 for DGE)
            idx16 = qpool.tile([P, SK], u16)
            idx32 = qpool.tile([P, SK], i32)
            nc.sync.dma_start(out=idx16[:], in_=idxb[:, :])
            nc.vector.tensor_scalar(
                out=idx32[:], in0=idx16[:], scalar1=0, scalar2=None, op0=ADD
            )
            deg_sb = cpool.tile([P, TILES_PER_CORE], f32)
            nc.sync.dma_start(out=deg_sb[:], in_=degt[:, :])

            # ---- phase A: project this core's shard; kv -> DRAM, q|skip -> SBUF
            def proj_slab(c0, w, wt, bt, sink):
                xt = pa.tile([P, 512], bf16, tag="xt")
                mn = pa.tile([P, 512], f32, tag="mn")
                yy = pa.tile([P, 512], f32, tag="yy")
                nc.sync.dma_start(out=xt[:, :w], in_=xcT[:, c0 : c0 + w])
                # y = elu(x)+1 = relu(x) + exp(min(x,0)); the +1 is folded
                # into the bias tiles host-side (b - colsum(W)).
                nc.vector.tensor_scalar(
                    out=mn[:, :w], in0=xt[:, :w], scalar1=0.0, scalar2=None, op0=MIN
                )
                nc.scalar.activation(out=yy[:, :w], in_=xt[:, :w], func=RELU)
                nc.scalar.activation(out=mn[:, :w], in_=mn[:, :w], func=EXP)
                nc.vector.tensor_add(out=yy[:, :w], in0=yy[:, :w], in1=mn[:, :w])
                for j in range(w // P):
                    ps = psA.tile([P, 2 * HC], f32, tag="ps")
                    nc.tensor.matmul(
                        out=ps[:],
                        lhsT=yy[:, j * P : (j + 1) * P],
                        rhs=wt[:],
                        start=True,
                        stop=True,
                    )
                    sink(c0 + j * P, ps, bt)

            def kv_sink(r0, ps, bt):
                kv_sb = pa.tile([P, 2 * HC], bf16, tag="kvsb")
                nc.vector.tensor_add(out=kv_sb[:], in0=ps[:], in1=bt[:])
                nc.sync.dma_start(out=kvsh[r0 : r0 + P, :], in_=kv_sb[:])

            def qs_sink(r0, ps, bt):
                nc.vector.tensor_add(
                    out=qs_sb[:, r0 : r0 + 2 * HC], in0=ps[:], in1=bt[:]
                )

            c0 = 0
            while c0 < NODES_PER_CORE:
                w = min(512, NODES_PER_CORE - c0)
                proj_slab(c0, w, wkv_sb, bkv_sb, kv_sink)
                c0 += w

            # ---- phase B: allgather the kv table (rank-ordered concat) ------
            nc.gpsimd.collective_compute(
                "AllGather",
                mybir.AluOpType.bypass,
                replica_groups=[list(range(NCORES))],
                ins=[kvsh[:].opt()],
                outs=[kvt[:].opt()],
            )

            # q|skip projections run on PE/vector while the collective flies
            c0 = 0
            while c0 < NODES_PER_CORE:
                w = min(512, NODES_PER_CORE - c0)
                proj_slab(c0, w, wqs_sb, bqs_sb, qs_sink)
                c0 += w

            # ---- phase C: gather + segment softmax + weighted sum ----------
            ot = 0
            for t in range(TILES_PER_CORE):
                K = int(k_per_tile[t])
                g_sb = pc.tile([P, K * D], bf16, tag="g")
                for k in range(K):
                    nc.gpsimd.indirect_dma_start(
                        out=g_sb[:, k * D : (k + 1) * D],
                        out_offset=None,
                        in_=kvt[:, :],
                        in_offset=IndirectOffsetOnAxis(
                            ap=idx32[:, ot + k : ot + k + 1], axis=0
                        ),
                    )
                # mask[p, e] = (e >= deg) * -1e30
                msk_sb = pcs.tile([P, KMAX], f32, tag="msk")
                nc.vector.tensor_scalar(
                    out=msk_sb[:, :K],
                    in0=iota_sb[:, :K],
                    scalar1=deg_sb[:, t : t + 1],
                    scalar2=NEG,
                    op0=ISGE,
                    op1=MULT,
                )
                ga = g_sb[:]
                pstr = ga.ap[0]
                goff = ga.offset
                # prod[p, h, e, c] = k_g[p, e, h, c] * q[p, h, c]
                prod = pcs.tile([P, 2 * K * C], f32, tag="prod")
                qsl = qs_sb[:, t * P : t * P + HC]
                nc.vector.tensor_tensor(
                    out=AP(
                        prod[:].tensor,
                        prod[:].offset,
                        [prod[:].ap[0], [C, K], [K * C, H], [1, C]],
                    ),
                    in0=AP(ga.tensor, goff, [pstr, [D, K], [C, H], [1, C]]),
                    in1=AP(qsl.tensor, qsl.offset, [qsl.ap[0], [0, K], [C, H], [1, C]]),
                    op=MULT,
                )
                alpha = pcs.tile([P, 2 * K], f32, tag="alpha")
                pv = prod[:]
                nc.vector.tensor_reduce(
                    out=alpha[:],
                    in_=AP(pv.tensor, pv.offset, [pv.ap[0], [K * C, H], [C, K], [1, C]]),
                    axis=AXX,
                    op=ADD,
                )
                mv = msk_sb[:]
                nc.vector.tensor_tensor(
                    out=alpha[:],
                    in0=alpha[:],
                    in1=AP(mv.tensor, mv.offset, [mv.ap[0], [0, H], [1, K]]),
                    op=ADD,
                )
                m_sb = pcs.tile([P, H], f32, tag="m")
                av = alpha[:]
                nc.vector.tensor_reduce(
                    out=m_sb[:],
                    in_=AP(av.tensor, av.offset, [av.ap[0], [K, H], [1, K]]),
                    axis=AXX,
                    op=MAX,
                )
                negm = pcs.tile([P, H], f32, tag="negm")
                nc.vector.tensor_scalar_mul(negm[:], m_sb[:], -1.0)
                ex = pcs.tile([P, 2 * K], f32, tag="ex")
                for h in range(H):
                    nc.scalar.activation(
                        out=ex[:, h * K : (h + 1) * K],
                        in_=alpha[:, h * K : (h + 1) * K],
                        func=EXP,
                        bias=negm[:, h : h + 1],
                        scale=1.0,
                    )
                den = pcs.tile([P, H], f32, tag="den")
                ev = ex[:]
                nc.vector.tensor_reduce(
                    out=den[:],
                    in_=AP(ev.tensor, ev.offset, [ev.ap[0], [K, H], [1, K]]),
                    axis=AXX,
                    op=ADD,
                )
                rden = pcs.tile([P, H], f32, tag="rden")
                nc.vector.reciprocal(rden[:], den[:])
                for h in range(H):
                    nc.vector.tensor_scalar(
                        out=ex[:, h * K : (h + 1) * K],
                        in0=ex[:, h * K : (h + 1) * K],
                        scalar1=rden[:, h : h + 1],
                        scalar2=None,
                        op0=MULT,
                    )
                # prod2[p, h, c, e] = v_g[p, e, h, c] * w[p, h, e]
                prod2 = pcs.tile([P, 2 * K * C], f32, tag="prod2")
                p2 = prod2[:]
                nc.vector.tensor_tensor(
                    out=AP(
                        p2.tensor, p2.offset, [p2.ap[0], [1, K], [K * C, H], [K, C]]
                    ),
                    in0=AP(ga.tensor, goff + HC, [pstr, [D, K], [C, H], [1, C]]),
                    in1=AP(ev.tensor, ev.offset, [ev.ap[0], [1, K], [K, H], [0, C]]),
                    op=MULT,
                )
                att = pcs.tile([P, HC], f32, tag="att")
                nc.vector.tensor_reduce(
                    out=att[:],
                    in_=AP(
                        p2.tensor, p2.offset, [p2.ap[0], [K * C, H], [K, C], [1, K]]
                    ),
                    axis=AXX,
                    op=ADD,
                )
                outt = pcs.tile([P, HC], bf16, tag="outt")
                nc.vector.tensor_add(
                    out=outt[:], in0=att[:], in1=qs_sb[:, t * P + HC : (t + 1) * P]
                )
                nc.sync.dma_start(out=outd[t * P : (t + 1) * P, :], in_=outt[:])
                ot += K
    _split_multi_waits(nc)
    return nc


# ---------------------------------------------------------------------------
def kernel(x, edge_index, Wq, bq, Wk, bk, Wv, bv, Wskip, bskip, _trace=False):
    x = np.asarray(x, np.float32)
    ei = np.asarray(edge_index)
    src = ei[0].astype(np.int64, copy=False)
    dst = ei[1].astype(np.int64, copy=False)
    Wq, bq, Wk, bk = (np.asarray(a, np.float32) for a in (Wq, bq, Wk, bk))
    Wv, bv, Wskip, bskip = (np.asarray(a, np.float32) for a in (Wv, bv, Wskip, bskip))

    s = np.float32(1.0 / np.sqrt(np.float32(C)))
    wkv_bf = np.concatenate([Wk, Wv], 1).astype(ml_dtypes.bfloat16)
    wqs_bf = np.concatenate([Wq * s, Wskip], 1).astype(ml_dtypes.bfloat16)
    # fold elu's +1 using the bf16-rounded weights actually used on device
    wkv32 = wkv_bf.astype(np.float32)
    wqs32 = wqs_bf.astype(np.float32)
    bkv = np.concatenate([bk, bv]) - wkv32.sum(0)
    bqs = np.concatenate([bq * s, bskip]) - wqs32.sum(0)
    bkv_t = np.tile(bkv.astype(np.float32)[None, :], (P, 1))
    bqs_t = np.tile(bqs.astype(np.float32)[None, :], (P, 1))

    # CSR over dst
    deg = np.bincount(dst, minlength=N).astype(np.int64)
    order = np.argsort(dst, kind="stable")
    src_sorted = src[order]
    dst_sorted = dst[order]
    rowptr = np.zeros(N + 1, np.int64)
    np.cumsum(deg, out=rowptr[1:])

    # degree-sorted nodes; sorted position i -> block g=i//P, lane p=i%P,
    # core c=g%8, tile t=g//8; allgather row = c*NODES_PER_CORE + t*P + p
    nodes_sorted = np.argsort(-deg, kind="stable")
    deg_sorted = deg[nodes_sorted]
    spos = np.empty(N, np.int64)
    spos[nodes_sorted] = np.arange(N)

    i_all = np.arange(NPAD, dtype=np.int64)
    g_all, p_all = i_all // P, i_all % P
    c_all, t_all = g_all % NCORES, g_all // NCORES
    kvpos_all = c_all * NODES_PER_CORE + t_all * P + p_all
    kvpos16 = kvpos_all[spos].astype(np.uint16)  # per real node

    k_per_tile = np.maximum(
        deg_sorted[np.arange(TILES_PER_CORE, dtype=np.int64) * (P * NCORES)], 2
    )
    SK = int(k_per_tile.sum())
    col_off = np.zeros(TILES_PER_CORE, np.int64)
    col_off[1:] = np.cumsum(k_per_tile)[:-1]

    # scatter every edge into its (core, partition, column) slot
    spos_d = spos[dst_sorted]
    p_e = spos_d % P
    g_e = spos_d // P
    c_e = g_e % NCORES
    t_e = g_e // NCORES
    r_e = np.arange(E, dtype=np.int64) - rowptr[dst_sorted]
    flat = (c_e * P + p_e) * SK + col_off[t_e] + r_e
    idx_all = np.zeros(NCORES * P * SK, np.uint16)
    idx_all[flat] = kvpos16[src_sorted]
    idx_all = idx_all.reshape(NCORES, P, SK)

    # per-(core, partition, tile) degrees for on-device masks
    degp = np.zeros(NPAD, np.int64)
    degp[:N] = deg_sorted
    degt_all = (
        degp.reshape(TILES_PER_CORE, NCORES, P).transpose(1, 2, 0).astype(np.float32)
    )

    # per-core x shards (permuted, transposed, bf16)
    nodes_pad = np.full(NPAD, -1, np.int64)
    nodes_pad[:N] = nodes_sorted
    core_nodes = (
        nodes_pad.reshape(TILES_PER_CORE, NCORES, P)
        .transpose(1, 0, 2)
        .reshape(NCORES, NODES_PER_CORE)
    )
    x_bf = x.astype(ml_dtypes.bfloat16)

    in_maps = []
    for c in range(NCORES):
        nodes = core_nodes[c]
        real = nodes >= 0
        xc = np.zeros((NODES_PER_CORE, D), ml_dtypes.bfloat16)
        xc[real] = x_bf[nodes[real]]
        in_maps.append(
            {
                "xcT": np.ascontiguousarray(xc.T),
                "wkv": wkv_bf,
                "wqs": wqs_bf,
                "bkv": bkv_t,
                "bqs": bqs_t,
                "idxb": np.ascontiguousarray(idx_all[c]),
                "degt": np.ascontiguousarray(degt_all[c]),
            }
        )

    nc = _build_program(k_per_tile)
    res = run_bass_kernel_spmd(nc, in_maps, core_ids=list(range(NCORES)))
    if _trace:
        # no NTFF hook in this container: report warm-NEFF wall time of a
        # second dispatch (upper bound: includes axon transfer + dispatch)
        import time as _time

        t0 = _time.time()
        res = run_bass_kernel_spmd(nc, in_maps, core_ids=list(range(NCORES)))
        kernel.last_wall_ns = int((_time.time() - t0) * 1e9)
    out_full = np.zeros((N, HC), np.float32)
    for c in range(NCORES):
        nodes = core_nodes[c]
        o = np.asarray(res.results[c]["out"], dtype=np.float32)
        real = nodes >= 0
        out_full[nodes[real]] = o[real]
    kernel.last_exec_time_ns = res.exec_time_ns
    return out_full


# revision 24
# speedup vs baseline: 6.5366x; 1.0987x over previous
"""TransformerConv graph attention (IntraGraphAttention) on 8 Trainium2 cores.

Sharding: nodes are degree-sorted and dealt block-cyclically (128-blocks) to 8
cores; each core receives ONLY its own x shard (bf16, transposed), computes
k|v projections for its shard, and the full k/v table is assembled on-device
with an AllGather over NeuronLink. Edges are sharded by dst node; per-edge
source positions ship as uint16, padding masks are built on-device from
per-node degrees. Output returns bf16. This keeps host<->device traffic at
~24MB total (vs ~260MB for a replicated-x design), which dominates wall time
through the PJRT transport.
"""

import sys

sys.path.insert(0, "/opt/trn_rl_repo")

import numpy as np
import ml_dtypes

import concourse.bass as bass
import concourse.mybir as mybir
import concourse.tile as tile
from concourse.bass import AP, IndirectOffsetOnAxis
from concourse.bass_utils import run_bass_kernel_spmd

N = 50000
E = 1_600_000
D = 128
H = 2
C = 32
HC = H * C  # 64
NCORES = 8
P = 128
NPAD = 50176          # 392 tiles of 128; per-core 49 tiles
TILES_PER_CORE = NPAD // P // NCORES  # 49
NODES_PER_CORE = TILES_PER_CORE * P   # 6272
NEG = -1.0e30

f32 = mybir.dt.float32
bf16 = mybir.dt.bfloat16
i32 = mybir.dt.int32
u16 = mybir.dt.uint16


# ---------------------------------------------------------------------------
# walrus in this container rejects instructions carrying >1 sync-wait; split
# extras onto same-engine NOPs (and the tail drain into single-wait drains).
def _patch_tile(tile_mod):
    from concourse.vector_clock import ScopedClock

    def _drain_and_barrier(self, tick_clock, wait_clock):
        nc = self.nc
        drain_inst = nc.sync.drain()
        wait_clock.add_sem_waits(
            drain_inst.ins, ScopedClock({None: tick_clock.global_clock})
        )
        si = drain_inst.ins.sync_info
        if si is not None and si.on_wait and len(si.on_wait) > 1:
            waits = list(si.on_wait)
            si.on_wait = waits[:1]
            for w in waits[1:]:
                extra = nc.sync.drain()
                esi = extra.ins.sync_info
                if esi is None:
                    extra.ins.sync_info = mybir.SyncInfo(on_wait=[w], on_update=[])
                else:
                    esi.on_wait = [w]
        nc.all_engine_barrier()
        assert self.sems is not None
        popped = nc._tile_sem_poison_stack.pop()
        assert popped is self._sem_poison
        nc.clear_and_free_semaphores(list(self.sems.allocated().values()))
        nc.all_engine_barrier()

    tile_mod.TileContext._drain_and_barrier = _drain_and_barrier


def _split_multi_waits(nc):
    f = nc.m.functions[0]
    for bb in f.blocks:
        out = []
        for inst in bb.instructions:
            si = inst.sync_info
            waits = list(si.on_wait) if (si is not None and si.on_wait) else []
            if len(waits) > 1:
                eng = inst.engine
                for w in waits[:-1]:
                    bi = nc.engines[eng].nop(nofuse=True)
                    mi = bi.ins
                    for b2 in f.blocks:
                        if mi in b2.instructions:
                            b2.instructions.remove(mi)
                            break
                    esi = mi.sync_info
                    if esi is None:
                        mi.sync_info = mybir.SyncInfo(on_wait=[w], on_update=[])
                    else:
                        esi.on_wait = [w]
                    out.append(mi)
                si.on_wait = waits[-1:]
            out.append(inst)
        bb.instructions[:] = out


_patch_tile(tile)


# ---------------------------------------------------------------------------
def _build_program(k_per_tile):
    """One SPMD program; per-core data differs but shapes are identical."""
    SK = int(sum(k_per_tile))
    KMAX = int(max(k_per_tile))
    nc = bass.Bass("TRN2", num_devices=NCORES)
    xcT = nc.dram_tensor("xcT", [P, NODES_PER_CORE], bf16, kind="ExternalInput")
    wkv = nc.dram_tensor("wkv", [D, 2 * HC], bf16, kind="ExternalInput")
    wqs = nc.dram_tensor("wqs", [D, 2 * HC], bf16, kind="ExternalInput")
    bkv = nc.dram_tensor("bkv", [1, 2 * HC], f32, kind="ExternalInput")
    bqs = nc.dram_tensor("bqs", [1, 2 * HC], f32, kind="ExternalInput")
    idxb = nc.dram_tensor("idxb", [P, SK], u16, kind="ExternalInput")
    degt = nc.dram_tensor("degt", [P, TILES_PER_CORE], f32, kind="ExternalInput")
    outd = nc.dram_tensor("out", [NODES_PER_CORE, HC], bf16, kind="ExternalOutput")
    kvsh = nc.dram_tensor("kvsh", [NODES_PER_CORE, D], bf16, kind="Internal")
    kvt = nc.dram_tensor(
        "kvt", [NPAD, D], bf16, kind="Internal", addr_space="Shared"
    )

    EXP = mybir.ActivationFunctionType.Exp
    RELU = mybir.ActivationFunctionType.Relu
    MIN = mybir.AluOpType.min
    MULT = mybir.AluOpType.mult
    ADD = mybir.AluOpType.add
    MAX = mybir.AluOpType.max
    ISGE = mybir.AluOpType.is_ge
    AXX = mybir.AxisListType.X

    with tile.TileContext(nc) as tc:
        with (
            tc.tile_pool(name="const", bufs=1) as cpool,
            tc.tile_pool(name="qs", bufs=1) as qpool,
            tc.tile_pool(name="pa", bufs=2) as pa,
            tc.tile_pool(name="psA", bufs=4, space="PSUM") as psA,
            tc.tile_pool(name="pc", bufs=2) as pc,
            tc.tile_pool(name="pcb", bufs=1) as pcb,
            tc.tile_pool(name="pcs", bufs=2) as pcs,
        ):
            wkv_bf = cpool.tile([D, 2 * HC], bf16)
            wqs_bf = cpool.tile([D, 2 * HC], bf16)
            wkv_sb = cpool.tile([D, 2 * HC], f32)
            wqs_sb = cpool.tile([D, 2 * HC], f32)
            bkv1 = cpool.tile([1, 2 * HC], f32)
            bqs1 = cpool.tile([1, 2 * HC], f32)
            ones1 = cpool.tile([1, P], f32)
            bkv_sb = cpool.tile([P, 2 * HC], f32)
            bqs_sb = cpool.tile([P, 2 * HC], f32)
            iota_sb = cpool.tile([P, KMAX], f32)
            nc.sync.dma_start(out=wkv_bf[:], in_=wkv[:, :])
            nc.sync.dma_start(out=wqs_bf[:], in_=wqs[:, :])
            nc.sync.dma_start(out=bkv1[:], in_=bkv[:, :])
            nc.sync.dma_start(out=bqs1[:], in_=bqs[:, :])
            nc.scalar.copy(out=wkv_sb[:], in_=wkv_bf[:])
            nc.scalar.copy(out=wqs_sb[:], in_=wqs_bf[:])
            nc.vector.memset(ones1[:], 1.0)
            # [1,128] biases -> all partitions via rank-1 matmul (outer product)
            psb = psA.tile([P, 2 * HC], f32, tag="psb")
            nc.tensor.matmul(out=psb[:], lhsT=ones1[:], rhs=bkv1[:], start=True, stop=True)
            nc.scalar.copy(out=bkv_sb[:], in_=psb[:])
            psb2 = psA.tile([P, 2 * HC], f32, tag="psb")
            nc.tensor.matmul(out=psb2[:], lhsT=ones1[:], rhs=bqs1[:], start=True, stop=True)
            nc.scalar.copy(out=bqs_sb[:], in_=psb2[:])
            nc.gpsimd.iota(
                iota_sb[:], pattern=[[1, KMAX]], base=0, channel_multiplier=0,
                allow_small_or_imprecise_dtypes=True,
            )
            qs_sb = qpool.tile([P, NODES_PER_CORE], f32)
            # full-size edge index (uint16 shipped, int32 for DGE)
            idx16 = qpool.tile([P, SK], u16)
            idx32 = qpool.tile([P, SK], i32)
            nc.sync.dma_start(out=idx16[:], in_=idxb[:, :])
            nc.vector.tensor_scalar(
                out=idx32[:], in0=idx16[:], scalar1=0, scalar2=None, op0=ADD
            )
            deg_sb = cpool.tile([P, TILES_PER_CORE], f32)
            nc.sync.dma_start(out=deg_sb[:], in_=degt[:, :])

            # ---- phase A: project this core's shard; kv -> DRAM, q|skip -> SBUF
            SLAB = 1024

            def proj_slab(c0, w, wt, bt, sink):
                xt = pa.tile([P, SLAB], bf16, tag="xt")
                mn = pa.tile([P, SLAB], f32, tag="mn")
                yy = pa.tile([P, SLAB], f32, tag="yy")
                nc.sync.dma_start(out=xt[:, :w], in_=xcT[:, c0 : c0 + w])
                # y = elu(x)+1 = relu(x) + exp(min(x,0)); the +1 is folded
                # into the bias tiles host-side (b - colsum(W)).
                nc.vector.tensor_scalar(
                    out=mn[:, :w], in0=xt[:, :w], scalar1=0.0, scalar2=None, op0=MIN
                )
                nc.scalar.activation(out=yy[:, :w], in_=xt[:, :w], func=RELU)
                nc.scalar.activation(out=mn[:, :w], in_=mn[:, :w], func=EXP)
                nc.vector.tensor_add(out=yy[:, :w], in0=yy[:, :w], in1=mn[:, :w])
                for j in range(w // P):
                    ps = psA.tile([P, 2 * HC], f32, tag="ps")
                    nc.tensor.matmul(
                        out=ps[:],
                        lhsT=yy[:, j * P : (j + 1) * P],
                        rhs=wt[:],
                        start=True,
                        stop=True,
                    )
                    sink(c0 + j * P, ps, bt)

            def kv_sink(r0, ps, bt):
                kv_sb = pa.tile([P, 2 * HC], bf16, tag="kvsb")
                nc.vector.tensor_add(out=kv_sb[:], in0=ps[:], in1=bt[:])
                nc.sync.dma_start(out=kvsh[r0 : r0 + P, :], in_=kv_sb[:])

            def qs_sink(r0, ps, bt):
                nc.vector.tensor_add(
                    out=qs_sb[:, r0 : r0 + 2 * HC], in0=ps[:], in1=bt[:]
                )

            c0 = 0
            while c0 < NODES_PER_CORE:
                w = min(SLAB, NODES_PER_CORE - c0)
                proj_slab(c0, w, wkv_sb, bkv_sb, kv_sink)
                c0 += w

            # ---- phase B: allgather the kv table (rank-ordered concat) ------
            nc.gpsimd.collective_compute(
                "AllGather",
                mybir.AluOpType.bypass,
                replica_groups=[list(range(NCORES))],
                ins=[kvsh[:].opt()],
                outs=[kvt[:].opt()],
            )

            # q|skip projections run on PE/vector while the collective flies
            c0 = 0
            while c0 < NODES_PER_CORE:
                w = min(SLAB, NODES_PER_CORE - c0)
                proj_slab(c0, w, wqs_sb, bqs_sb, qs_sink)
                c0 += w

            # one [P, TILES*HC] staging buffer; single batched output DMA
            out_sb = qpool.tile([P, TILES_PER_CORE * HC], bf16)

            # ---- phase C: gather + segment softmax + weighted sum ----------
            # multi-index DGE gather: 8 uint32 offsets per partition per
            # instruction (f32 rows only; >8 or non-f32 silently drops all
            # but the first 128 indices — probed on HW).
            ot = 0
            for t in range(TILES_PER_CORE):
                K = int(k_per_tile[t])
                g_sb = pc.tile([P, K * D], bf16, tag="g")
                # one [P,1]-offset indirect DMA per slot: the DGE multi-index
                # mode only works for consecutive (coalescable) row indices,
                # so arbitrary gathers must stay at 128 offsets/instruction.
                for k in range(K):
                    nc.gpsimd.indirect_dma_start(
                        out=g_sb[:, k * D : (k + 1) * D],
                        out_offset=None,
                        in_=kvt[:, :],
                        in_offset=IndirectOffsetOnAxis(
                            ap=idx32[:, ot + k : ot + k + 1], axis=0
                        ),
                    )
                # mask[p, e] = (e >= deg) * -1e30
                msk_sb = pcs.tile([P, KMAX], f32, tag="msk")
                nc.vector.tensor_scalar(
                    out=msk_sb[:, :K],
                    in0=iota_sb[:, :K],
                    scalar1=deg_sb[:, t : t + 1],
                    scalar2=NEG,
                    op0=ISGE,
                    op1=MULT,
                )
                ga = g_sb[:]
                pstr = ga.ap[0]
                goff = ga.offset
                # prod[p, h, e, c] = k_g[p, e, h, c] * q[p, h, c]
                prod = pcb.tile([P, 2 * K * C], f32, tag="prod")
                qsl = qs_sb[:, t * P : t * P + HC]
                nc.vector.tensor_tensor(
                    out=AP(
                        prod[:].tensor,
                        prod[:].offset,
                        [prod[:].ap[0], [C, K], [K * C, H], [1, C]],
                    ),
                    in0=AP(ga.tensor, goff, [pstr, [D, K], [C, H], [1, C]]),
                    in1=AP(qsl.tensor, qsl.offset, [qsl.ap[0], [0, K], [C, H], [1, C]]),
                    op=MULT,
                )
                alpha = pcs.tile([P, 2 * K], f32, tag="alpha")
                pv = prod[:]
                nc.vector.tensor_reduce(
                    out=alpha[:],
                    in_=AP(pv.tensor, pv.offset, [pv.ap[0], [K * C, H], [C, K], [1, C]]),
                    axis=AXX,
                    op=ADD,
                )
                mv = msk_sb[:]
                nc.vector.tensor_tensor(
                    out=alpha[:],
                    in0=alpha[:],
                    in1=AP(mv.tensor, mv.offset, [mv.ap[0], [0, H], [1, K]]),
                    op=ADD,
                )
                m_sb = pcs.tile([P, H], f32, tag="m")
                av = alpha[:]
                nc.vector.tensor_reduce(
                    out=m_sb[:],
                    in_=AP(av.tensor, av.offset, [av.ap[0], [K, H], [1, K]]),
                    axis=AXX,
                    op=MAX,
                )
                negm = pcs.tile([P, H], f32, tag="negm")
                nc.vector.tensor_scalar_mul(negm[:], m_sb[:], -1.0)
                ex = pcs.tile([P, 2 * K], f32, tag="ex")
                for h in range(H):
                    nc.scalar.activation(
                        out=ex[:, h * K : (h + 1) * K],
                        in_=alpha[:, h * K : (h + 1) * K],
                        func=EXP,
                        bias=negm[:, h : h + 1],
                        scale=1.0,
                    )
                den = pcs.tile([P, H], f32, tag="den")
                ev = ex[:]
                nc.vector.tensor_reduce(
                    out=den[:],
                    in_=AP(ev.tensor, ev.offset, [ev.ap[0], [K, H], [1, K]]),
                    axis=AXX,
                    op=ADD,
                )
                rden = pcs.tile([P, H], f32, tag="rden")
                nc.vector.reciprocal(rden[:], den[:])
                for h in range(H):
                    nc.vector.tensor_scalar(
                        out=ex[:, h * K : (h + 1) * K],
                        in0=ex[:, h * K : (h + 1) * K],
                        scalar1=rden[:, h : h + 1],
                        scalar2=None,
                        op0=MULT,
                    )
                # prod2[p, h, c, e] = v_g[p, e, h, c] * w[p, h, e]
                prod2 = pcb.tile([P, 2 * K * C], f32, tag="prod2")
                p2 = prod2[:]
                nc.vector.tensor_tensor(
                    out=AP(
                        p2.tensor, p2.offset, [p2.ap[0], [1, K], [K * C, H], [K, C]]
                    ),
                    in0=AP(ga.tensor, goff + HC, [pstr, [D, K], [C, H], [1, C]]),
                    in1=AP(ev.tensor, ev.offset, [ev.ap[0], [1, K], [K, H], [0, C]]),
                    op=MULT,
                )
                att = pcs.tile([P, HC], f32, tag="att")
                nc.vector.tensor_reduce(
                    out=att[:],
                    in_=AP(
                        p2.tensor, p2.offset, [p2.ap[0], [K * C, H], [K, C], [1, K]]
                    ),
                    axis=AXX,
                    op=ADD,
                )
                nc.vector.tensor_add(
                    out=out_sb[:, t * HC : (t + 1) * HC],
                    in0=att[:],
                    in1=qs_sb[:, t * P + HC : (t + 1) * P],
                )
                ot += K
            # one batched output DMA: out_sb[p, t*HC+c] -> outd[t*P+p, c]
            ov = out_sb[:]
            nc.sync.dma_start(
                out=AP(outd, 0, [[HC, P], [P * HC, TILES_PER_CORE], [1, HC]]),
                in_=AP(ov.tensor, ov.offset, [ov.ap[0], [HC, TILES_PER_CORE], [1, HC]]),
            )
    _split_multi_waits(nc)
    return nc


# ---------------------------------------------------------------------------
def kernel(x, edge_index, Wq, bq, Wk, bk, Wv, bv, Wskip, bskip, _trace=False):
    x = np.asarray(x, np.float32)
    ei = np.asarray(edge_index)
    src = ei[0].astype(np.int64, copy=False)
    dst = ei[1].astype(np.int64, copy=False)
    Wq, bq, Wk, bk = (np.asarray(a, np.float32) for a in (Wq, bq, Wk, bk))
    Wv, bv, Wskip, bskip = (np.asarray(a, np.float32) for a in (Wv, bv, Wskip, bskip))

    s = np.float32(1.0 / np.sqrt(np.float32(C)))
    wkv_bf = np.concatenate([Wk, Wv], 1).astype(ml_dtypes.bfloat16)
    wqs_bf = np.concatenate([Wq * s, Wskip], 1).astype(ml_dtypes.bfloat16)
    # fold elu's +1 using the bf16-rounded weights actually used on device
    wkv32 = wkv_bf.astype(np.float32)
    wqs32 = wqs_bf.astype(np.float32)
    bkv = (np.concatenate([bk, bv]) - wkv32.sum(0)).astype(np.float32)[None, :]
    bqs = (np.concatenate([bq * s, bskip]) - wqs32.sum(0)).astype(np.float32)[None, :]

    # CSR over dst
    deg = np.bincount(dst, minlength=N).astype(np.int64)
    order = np.argsort(dst, kind="stable")
    src_sorted = src[order]
    dst_sorted = dst[order]
    rowptr = np.zeros(N + 1, np.int64)
    np.cumsum(deg, out=rowptr[1:])

    # degree-sorted nodes; sorted position i -> block g=i//P, lane p=i%P,
    # core c=g%8, tile t=g//8; allgather row = c*NODES_PER_CORE + t*P + p
    nodes_sorted = np.argsort(-deg, kind="stable")
    deg_sorted = deg[nodes_sorted]
    spos = np.empty(N, np.int64)
    spos[nodes_sorted] = np.arange(N)

    i_all = np.arange(NPAD, dtype=np.int64)
    g_all, p_all = i_all // P, i_all % P
    c_all, t_all = g_all % NCORES, g_all // NCORES
    kvpos_all = c_all * NODES_PER_CORE + t_all * P + p_all
    kvpos16 = kvpos_all[spos].astype(np.uint16)  # per real node

    k_per_tile = np.maximum(
        deg_sorted[np.arange(TILES_PER_CORE, dtype=np.int64) * (P * NCORES)], 2
    )
    SK = int(k_per_tile.sum())
    col_off = np.zeros(TILES_PER_CORE, np.int64)
    col_off[1:] = np.cumsum(k_per_tile)[:-1]

    # scatter every edge into its (core, partition, column) slot
    spos_d = spos[dst_sorted]
    p_e = spos_d % P
    g_e = spos_d // P
    c_e = g_e % NCORES
    t_e = g_e // NCORES
    r_e = np.arange(E, dtype=np.int64) - rowptr[dst_sorted]
    flat = (c_e * P + p_e) * SK + col_off[t_e] + r_e
    idx_all = np.zeros(NCORES * P * SK, np.uint16)
    idx_all[flat] = kvpos16[src_sorted]
    idx_all = idx_all.reshape(NCORES, P, SK)

    # per-(core, partition, tile) degrees for on-device masks
    degp = np.zeros(NPAD, np.int64)
    degp[:N] = deg_sorted
    degt_all = (
        degp.reshape(TILES_PER_CORE, NCORES, P).transpose(1, 2, 0).astype(np.float32)
    )

    # per-core x shards (permuted, transposed, bf16)
    nodes_pad = np.full(NPAD, -1, np.int64)
    nodes_pad[:N] = nodes_sorted
    core_nodes = (
        nodes_pad.reshape(TILES_PER_CORE, NCORES, P)
        .transpose(1, 0, 2)
        .reshape(NCORES, NODES_PER_CORE)
    )
    x_bf = x.astype(ml_dtypes.bfloat16)

    in_maps = []
    for c in range(NCORES):
        nodes = core_nodes[c]
        real = nodes >= 0
        xc = np.zeros((NODES_PER_CORE, D), ml_dtypes.bfloat16)
        xc[real] = x_bf[nodes[real]]
        in_maps.append(
            {
                "xcT": np.ascontiguousarray(xc.T),
                "wkv": wkv_bf,
                "wqs": wqs_bf,
                "bkv": bkv,
                "bqs": bqs,
                "idxb": np.ascontiguousarray(idx_all[c]),
                "degt": np.ascontiguousarray(degt_all[c]),
            }
        )

    nc = _build_program(k_per_tile)
    res = run_bass_kernel_spmd(nc, in_maps, core_ids=list(range(NCORES)))
    if _trace:
        # no NTFF hook in this container: report warm-NEFF wall time of a
        # second dispatch (upper bound: includes axon transfer + dispatch)
        import time as _time

        t0 = _time.time()
        res = run_bass_kernel_spmd(nc, in_maps, core_ids=list(range(NCORES)))
        kernel.last_wall_ns = int((_time.time() - t0) * 1e9)
    out_full = np.zeros((N, HC), np.float32)
    for c in range(NCORES):
        nodes = core_nodes[c]
        o = np.asarray(res.results[c]["out"], dtype=np.float32)
        real = nodes >= 0
        out_full[nodes[real]] = o[real]
    kernel.last_exec_time_ns = res.exec_time_ns
    return out_full
